# revision 1
# baseline (speedup 1.0000x reference)
"""CRF loss (forward-algorithm log-partition + gold-path score) on 8 Trainium2
NeuronCores.

Algorithm (per batch row):
  log_den = logsumexp over tag paths (forward recursion over S=512 steps)
  log_num = score of the gold tag path
  loss    = mean_b(log_den - log_num)

Device mapping:
  * Linear-space forward recursion:  q_{t+1} = (E_bd @ q_t) * exp(e_t - c0)
    with E_bd = block-diag(exp(transition)), c0 = 6*ln2 a constant rescale
    that keeps q inside fp32/bf16 exponent range.  At the end
    log_den = ln(sum_j q) + S*c0.  This is the only serial part: 511 rounds
    of one 128x128 block-diag matmul (PE) + one PSUM*SBUF multiply (DVE)
    per 128-row chain, two independent chains pipelined per core.
  * Gold score is computed in bulk (no serial chain, no gathers):
      gold_total = sum_{s,j} hp[j,s] * (e[j,s] + V[j,s-1]),
      V = blockdiag(trans) ^T-applied to the one-hot stream:
      V[:, s] = trans[tag_s, :]  via matmuls with the one-hot tensor.
    Products+reductions run as a few large fused ops (PE matmuls, GPSIMD
    adds, DVE tensor_tensor_reduce) fully overlapped with the recursion.
  * Data-parallel over batch: each core takes 256 rows as 2 chains of 128
    rows; each chain packs 4 groups of 32 rows into the 128 partitions with
    the (padded) 32-wide tag dim per group, so the 24x24 tag contraction is
    a single 128x128 block-diagonal matmul per step per chain.

Host side only reshapes/pads/one-hot-encodes inputs into the packed
[group*32+tag, step*32+row] layout; all arithmetic of the loss runs on
device.
"""

import math
import os

import numpy as np
import ml_dtypes

import concourse.bass as bass
import concourse.bacc as bacc
import concourse.tile as tile
import concourse.mybir as mybir
import concourse.bass_utils as bass_utils
from concourse.bass_utils import run_bass_kernel_spmd

BF16 = mybir.dt.bfloat16
F32 = mybir.dt.float32
AF = mybir.ActivationFunctionType
ALU = mybir.AluOpType
NPBF16 = ml_dtypes.bfloat16

B, S, NT = 2048, 512, 24
JP = 32                    # padded tag dim (multiple of 32)
NCORES = 8
RPC = B // NCORES          # rows per core (256)
NCHAIN = 2                 # chains per core, 128 rows each
CS = 64                    # steps per prep chunk
C0 = 6.0 * math.log(2.0)   # per-step rescale of the partition chain
NEG = -30000.0             # pad value; exp -> 0
VMM_FD = 256               # free-dim per gold V matmul
VMM_PER_CHUNK = CS * JP // VMM_FD  # 4


def build_program(s_total=S):
    assert s_total % CS == 0
    nch = s_total // CS
    nc = bacc.Bacc(trn_type="TRN2")
    ep_d = nc.dram_tensor("ep", [RPC, s_total * JP], BF16, kind="ExternalInput")
    hp_d = nc.dram_tensor("hp", [RPC, s_total * JP], BF16, kind="ExternalInput")
    tr_d = nc.dram_tensor("tr", [NT, NT], F32, kind="ExternalInput")
    out_d = nc.dram_tensor("out", [1, 1], F32, kind="ExternalOutput")

    with tile.TileContext(nc) as tc:
        with tc.tile_pool(name="const", bufs=1) as const, \
             tc.tile_pool(name="chunks", bufs=3) as chunks, \
             tc.tile_pool(name="state", bufs=1) as state, \
             tc.tile_pool(name="small", bufs=4) as small, \
             tc.tile_pool(name="pmm", bufs=2, space="PSUM") as pmm, \
             tc.tile_pool(name="pv", bufs=3, space="PSUM") as pv, \
             tc.tile_pool(name="psg", bufs=1, space="PSUM") as psg:

            # ---- constants (memset-only parts; DMA parts deferred) ----
            ebd = const.tile([128, 128], BF16)
            nc.vector.memset(ebd[:, :], 0.0)
            tbd = const.tile([128, 128], BF16)
            nc.vector.memset(tbd[:, :], 0.0)
            osel = const.tile([128, 4], BF16)
            nc.vector.memset(osel[:, :], 0.0)
            for g in range(4):
                nc.vector.memset(osel[32 * g:32 * g + 32, g:g + 1], 1.0)
            ones4 = const.tile([4, 1], F32)
            nc.vector.memset(ones4[:, :], 1.0)
            onesn128 = const.tile([128, 1], F32)
            nc.vector.memset(onesn128[:, :], -1.0)
            negc0 = const.tile([128, 1], F32)
            nc.vector.memset(negc0[:, :], -C0)

            # ---- persistent per-chain state ----
            q = [state.tile([128, JP], BF16, name=f"q{c}")
                 for c in range(NCHAIN)]
            gparts = []
            for c in range(NCHAIN):
                g_t = state.tile([128, 2 * nch], F32, name=f"gparts{c}")
                nc.vector.memset(g_t[:, :], 0.0)
                gparts.append(g_t)

            ep_tiles = [[None] * nch for _ in range(NCHAIN)]
            a1_tiles = [dict() for _ in range(NCHAIN)]
            prod_tiles = [dict() for _ in range(NCHAIN)]
            hp_tiles = [[None] * nch for _ in range(NCHAIN)]
            fh_tiles = [[None] * nch for _ in range(NCHAIN)]
            v_tiles = [[None] * nch for _ in range(NCHAIN)]

            def prep_chunk(c, k):
                ep_t = chunks.tile([128, CS, JP], BF16, tag=f"ep{c}",
                                   name=f"ep{c}_{k}")
                hp_t = chunks.tile([128, CS, JP], BF16, tag=f"hp{c}",
                                   name=f"hp{c}_{k}")
                lo = k * CS * JP
                nc.sync.dma_start(
                    out=ep_t[:, :, :],
                    in_=ep_d[c * 128:(c + 1) * 128, lo:lo + CS * JP]
                    .rearrange("p (s j) -> p s j", j=JP))
                nc.sync.dma_start(
                    out=hp_t[:, :, :],
                    in_=hp_d[c * 128:(c + 1) * 128, lo:lo + CS * JP]
                    .rearrange("p (s j) -> p s j", j=JP))
                fh = chunks.tile([128, CS, JP], BF16, tag=f"fh{c}",
                                 name=f"fh{c}_{k}")
                # F_hat = exp(ep - c0); split so early steps unblock sooner
                for e4 in range(4):
                    ssl = slice(e4 * CS // 4, (e4 + 1) * CS // 4)
                    nc.scalar.activation(fh[:, ssl, :], ep_t[:, ssl, :],
                                         AF.Exp, bias=negc0[:, :])
                ep_tiles[c][k] = ep_t
                hp_tiles[c][k] = hp_t
                fh_tiles[c][k] = fh

            # ---- gold bulk ops (emitted sprinkled through chunk k's steps) --
            def get_vtile(c, k):
                if v_tiles[c][k] is None:
                    v_tiles[c][k] = chunks.tile(
                        [128, (CS + 1) * JP], BF16, tag=f"v{c}",
                        name=f"v{c}_{k}")
                return v_tiles[c][k]

            def gold_vmm(c, k, i):
                # V[:, JP + i*FD : JP + (i+1)*FD] = tbd.T @ hp-slice
                get_vtile(c, k)
                vp = pv.tile([128, VMM_FD], F32, tag="vp", name=f"vp{c}_{k}_{i}")
                hp_t = hp_tiles[c][k]
                nc.tensor.matmul(
                    vp[:, :], tbd[:, :],
                    hp_t[:, :, :].rearrange("p s j -> p (s j)")
                    [:, i * VMM_FD:(i + 1) * VMM_FD],
                    start=True, stop=True)
                nc.scalar.copy(
                    v_tiles[c][k][:, JP + i * VMM_FD: JP + (i + 1) * VMM_FD],
                    vp[:, :])

            def gold_carry(c, k):
                v_t = get_vtile(c, k)
                if k == 0:
                    nc.vector.memset(v_t[:, 0:JP], 0.0)
                else:
                    nc.vector.tensor_copy(v_t[:, 0:JP],
                                          v_tiles[c][k - 1][:, CS * JP:(CS + 1) * JP])

            GH = CS * JP // 4  # 512-col quarter-chunk slices

            def gold_add(c, k, h):
                # A1 = ep + V_backshift (DVE, sliced to ride round slack)
                if k not in a1_tiles[c]:
                    a1_tiles[c][k] = chunks.tile([128, CS * JP], BF16,
                                                 tag="a1", name=f"a1_{c}_{k}")
                sel = slice(h * GH, (h + 1) * GH)
                nc.vector.tensor_add(
                    a1_tiles[c][k][:, sel],
                    ep_tiles[c][k][:, :, :].rearrange("p s j -> p (s j)")[:, sel],
                    v_tiles[c][k][:, 0:CS * JP][:, sel])

            def gold_mul(c, k, h):
                if k not in prod_tiles[c]:
                    prod_tiles[c][k] = chunks.tile([128, CS * JP], BF16,
                                                   tag="prod",
                                                   name=f"prod_{c}_{k}")
                sel = slice(h * GH, (h + 1) * GH)
                hp_flat = hp_tiles[c][k][:, :, :].rearrange("p s j -> p (s j)")
                nc.vector.tensor_mul(prod_tiles[c][k][:, sel],
                                     a1_tiles[c][k][:, sel], hp_flat[:, sel])

            def gold_red(c, k, hh):
                # reduce a 1024-col half of prod on ScalarE (off the DVE)
                sel = slice(hh * 2 * GH, (hh + 1) * 2 * GH)
                scr = chunks.tile([128, 2 * GH], BF16, tag="scr",
                                  name=f"scr_{c}_{k}_{hh}")
                nc.scalar.activation(scr[:, :], prod_tiles[c][k][:, sel],
                                     AF.Copy,
                                     accum_out=gparts[c][:, 2 * k + hh:
                                                         2 * k + hh + 1])

            # ---- pipeline ----
            for c in range(NCHAIN):
                prep_chunk(c, 0)
            if nch > 1:
                for c in range(NCHAIN):
                    prep_chunk(c, 1)
            # deferred constant builds (their small DMAs queue after the
            # big chunk-0 loads that gate the first recursion rounds)
            tr32 = const.tile([NT, NT], F32)
            nc.sync.dma_start(out=tr32[:, :], in_=tr_d[:, :])
            e24 = const.tile([NT, NT], BF16)
            nc.scalar.activation(e24[:, :], tr32[:, :], AF.Exp)
            t24 = const.tile([NT, NT], BF16)
            nc.vector.tensor_copy(t24[:, :], tr32[:, :])
            for g in range(4):
                # SBUF->SBUF DMA to place blocks on the diagonal
                nc.sync.dma_start(
                    out=ebd[32 * g:32 * g + NT, 32 * g:32 * g + NT],
                    in_=e24[:, :])
                nc.sync.dma_start(
                    out=tbd[32 * g:32 * g + NT, 32 * g:32 * g + NT],
                    in_=t24[:, :])
            for c in range(NCHAIN):
                nc.vector.tensor_copy(q[c][:, :], fh_tiles[c][0][:, 0, :])

            for k in range(nch):
                if k + 2 < nch:
                    for c in range(NCHAIN):
                        prep_chunk(c, k + 2)
                s_lo = 1 if k == 0 else 0
                for sl in range(s_lo, CS):
                    for c in range(NCHAIN):
                        p_t = pmm.tile([128, JP], F32, tag=f"p{c}",
                                       name=f"p{c}_{k}_{sl}")
                        nc.tensor.matmul(p_t[:, :], ebd[:, :], q[c][:, :],
                                         start=True, stop=True)
                        nc.vector.tensor_mul(q[c][:, :], p_t[:, :],
                                             fh_tiles[c][k][:, sl, :])
                    # sprinkle the bulk gold work between recursion rounds,
                    # staggered per chain to avoid paired stalls
                    if sl == 2:
                        for c in range(NCHAIN):
                            gold_carry(c, k)
                    for c in range(NCHAIN):
                        base = 3 + 2 * c
                        if (sl - base) % 4 == 0:
                            i = (sl - base) // 4
                            if 0 <= i < VMM_PER_CHUNK:
                                gold_vmm(c, k, i)
                    # add piece h of chain c needs vmm i=2h+1 (done at
                    # sl 3+2c+4(2h+1)); mul piece h needs add piece h.
                    _ADD = {9: (0, 0), 13: (1, 0), 17: (0, 1), 21: (1, 1),
                            25: (0, 2), 29: (1, 2), 33: (0, 3), 37: (1, 3)}
                    _MUL = {11: (0, 0), 15: (1, 0), 41: (0, 1), 45: (1, 1),
                            49: (0, 2), 53: (1, 2), 57: (0, 3), 61: (1, 3)}
                    _RED = {43: (0, 0), 47: (1, 0), 59: (0, 1), 63: (1, 1)}
                    if sl in _ADD:
                        gold_add(c=_ADD[sl][0], k=k, h=_ADD[sl][1])
                    elif sl in _MUL:
                        gold_mul(c=_MUL[sl][0], k=k, h=_MUL[sl][1])
                    if sl in _RED:
                        gold_red(c=_RED[sl][0], k=k, hh=_RED[sl][1])

            # ---- finalization ----
            ldr = []
            for c in range(NCHAIN):
                tq = psg.tile([4, JP], F32, tag="tg", name=f"tq{c}")
                nc.tensor.matmul(tq[:, :], osel[:, :], q[c][:, :],
                                 start=True, stop=True)
                ld = small.tile([4, JP], F32, tag="ld", name=f"ld{c}")
                nc.scalar.activation(ld[:, :], tq[:, :], AF.Ln)
                r_t = small.tile([4, 1], F32, tag="rs", name=f"rs{c}")
                nc.vector.reduce_sum(r_t[:, :], ld[:, :],
                                     axis=mybir.AxisListType.X)
                ldr.append(r_t)
            lsum = small.tile([4, 1], F32, tag="lsum", name="lsum")
            nc.vector.tensor_add(lsum[:, :], ldr[0][:, :], ldr[1][:, :])

            gr = []
            for c in range(NCHAIN):
                g_t = small.tile([128, 1], F32, tag="gr", name=f"gr{c}")
                nc.vector.reduce_sum(g_t[:, :], gparts[c][:, :],
                                     axis=mybir.AxisListType.X)
                gr.append(g_t)
            gsum = small.tile([128, 1], F32, tag="gsum", name="gsum")
            nc.vector.tensor_add(gsum[:, :], gr[0][:, :], gr[1][:, :])

            finl = psg.tile([1, 1], F32, tag="tg", name="finl")
            nc.tensor.matmul(finl[:, :], ones4[:, :], lsum[:, :],
                             start=True, stop=True)
            fing = psg.tile([1, 1], F32, tag="tg", name="fing")
            nc.tensor.matmul(fing[:, :], onesn128[:, :], gsum[:, :],
                             start=True, stop=True)
            sl_t = small.tile([1, 1], F32, tag="outv", name="sl_t")
            nc.scalar.copy(sl_t[:, :], finl[:, :])
            sg_t = small.tile([1, 1], F32, tag="outv", name="sg_t")
            nc.scalar.copy(sg_t[:, :], fing[:, :])
            outv = small.tile([1, 1], F32, tag="outv", name="outv")
            # + per-core constant: RPC rows * S steps * c0  (sg holds -gold)
            nc.vector.scalar_tensor_tensor(
                outv[:, :], sl_t[:, :], float(RPC) * float(s_total) * C0,
                sg_t[:, :], ALU.add, ALU.add)
            nc.sync.dma_start(out=out_d[:, :], in_=outv[:, :])
    _bacc_compile_no_ldw_split(nc)
    return nc


def _bacc_compile_no_ldw_split(nc):
    """Bacc.compile() minus move_matmul_waits_to_ldweights (so our
    Ldweights-dedup below stays valid; generate_event_semaphores handles
    multi-wait matmuls)."""
    from concourse import inst_simplify

    nc.insert_bir_kernel_barrier_sem_inc()
    nc.generate_event_semaphores()
    nc.remove_dead_instructions_after_branch()
    nc.validate_blocks()
    nc.dce_regs()
    nc.thread_jumps()
    nc.remove_dead_blocks()
    nc.remove_dead_allocations()
    nc.verify_switch_hints()
    nc.alloc_regs()
    inst_simplify.simplify(nc)
    nc.fuse_regops()
    nc.fuse_blocks()
    nc.replace_nops_with_events()
    for engine in nc.engines:
        nc.fuse_nops(engine)
    nc.remove_dead_nops()
    nc.remove_dangling_data()
    nc.generate_event_semaphores()
    nc.insert_library_loads()
    nc.insert_act_table_loads()
    nc.insert_hostgen_rebases()
    nc.codegen_inst_isa_subclasses()
    _dedup_ldweights(nc)


def _dedup_ldweights(nc):
    """Drop PE Ldweights that reload the already-loaded stationary weights.

    codegen_inst_isa_subclasses splits every matmul into Ldweights+Matmult;
    long runs of recursion matmuls share one stationary matrix, so the
    repeated 128-row reload (~100ns each) would clog the PE stream.  Keep
    any Ldweights carrying sync waits/updates to preserve semaphores."""
    removed = 0
    for fn in nc.m.functions:
        for blk in fn.blocks:
            cur_sig = None
            out = []
            for inst in blk.instructions:
                tname = type(inst).__name__
                if tname == "InstLdweights":
                    sig = inst.concise().split("Ldweights", 1)[-1]
                    if sig == cur_sig and not inst.has_wait() \
                            and not inst.has_update():
                        removed += 1
                        continue
                    cur_sig = sig
                elif tname in ("InstMatmult", "InstMatmultMx"):
                    pass  # uses loaded weights; state unchanged
                elif str(inst.engine) == "EngineType.PE" and tname not in (
                        "InstEventSemaphore", "InstNop", "InstSemWait"):
                    cur_sig = None
                out.append(inst)
            blk.instructions = out
    return removed


def prep_inputs(emissions, tags, s_total=S):
    """Host-side layout prep: per-core packed-transposed bf16 tensors.

    ep[core][chain*128 + G*32 + j, s*32 + b] = emissions[row, s, j] (pad NEG)
    hp likewise one-hot(tags) in {0, 1}.
    row = core*256 + chain*128 + G*32 + b.
    """
    e = np.asarray(emissions)[:, :s_total, :]
    t = np.asarray(tags)[:, :s_total]
    eb = e.astype(NPBF16)
    ep = np.full((B, s_total, JP), NEG, dtype=NPBF16)
    ep[:, :, :NT] = eb
    hp = np.zeros((B, s_total, JP), dtype=NPBF16)
    np.put_along_axis(hp, t[..., None], np.asarray(1.0, NPBF16), axis=2)

    def pack(x):
        # [B, s, JP] -> [cores, chain, G, b, s, j] -> [cores, chain, G, j, s, b]
        x = x.reshape(NCORES, NCHAIN, 4, 32, s_total, JP)
        x = np.ascontiguousarray(x.transpose(0, 1, 2, 5, 4, 3))
        return x.reshape(NCORES, RPC, s_total * JP)

    return pack(ep), pack(hp)


_PROGRAM_CACHE = {}


def kernel(emissions, tags, mask, transition_scores):
    assert np.asarray(mask).min() == 1, "kernel assumes all-ones mask"
    ep, hp = prep_inputs(emissions, tags)
    tr = np.ascontiguousarray(np.asarray(transition_scores, dtype=np.float32))

    if S not in _PROGRAM_CACHE:
        _PROGRAM_CACHE[S] = build_program(S)
    nc = _PROGRAM_CACHE[S]

    in_maps = [
        {"ep": np.ascontiguousarray(ep[c]), "hp": np.ascontiguousarray(hp[c]),
         "tr": tr}
        for c in range(NCORES)
    ]
    res = run_bass_kernel_spmd(nc, in_maps, core_ids=list(range(NCORES)))
    partials = [float(r["out"][0, 0]) for r in res.results]
    return np.float32(sum(partials) / B)



# revision 24
# speedup vs baseline: 1.3085x; 1.3085x over previous
"""CRF loss (forward-algorithm log-partition + gold-path score) on 8 Trainium2
NeuronCores.

v2 design — bidirectional recursion, 256 serial slots instead of 511:

  log_den per row: split the 512-step forward algorithm at the midpoint.
    fwd:  q_0 = f_0;          q_t = f_t * (E^T q_{t-1})      t = 1..255
    bwd:  g_511 = f_511;      g_t = f_t * (E   g_{t+1})      t = 510..256
    p_256 = E^T q_255
    log_den = ln(g_256 . p_256) + 512*c0,   f_t = exp(e_t - c0)
  Both directions run in the SAME per-slot matmul via a block-diagonal
  128x128 stationary: partition groups 0-1 hold E-blocks (fwd), groups
  2-3 hold E^T-blocks (bwd).  Per core: 2 chains x 128 rows, each chain
  state is [128 part, 64 cols] (fwd+bwd of 128 rows).  Per slot per
  chain: one matmul (N=64) + one DVE multiply.  255 mul slots + 1
  stitch matmul.

  Gold score: emission part = sum(hp * ep) via DVE tensor_tensor_reduce
  over the same packed layout (hp = one-hot tags); transition part is a
  host-packed lookup tensor trg[row, s] = trans[tag_{s-1}, tag_s]
  (pure indexing of tags into the tiny 24x24 table), summed on ACT.

  PE is kept streaming with always-ready filler matmuls so recursion
  matmuls overlap fill/drain instead of paying isolated latency.
"""

import math
import os

import numpy as np
import ml_dtypes

import concourse.bass as bass
import concourse.bacc as bacc
import concourse.tile as tile
import concourse.mybir as mybir
import concourse.bass_utils as bass_utils
from concourse.bass_utils import run_bass_kernel_spmd

BF16 = mybir.dt.bfloat16
F32 = mybir.dt.float32
AF = mybir.ActivationFunctionType
ALU = mybir.AluOpType
NPBF16 = ml_dtypes.bfloat16

B, S, NT = 2048, 512, 24
JP = 32                    # padded tag dim
NCORES = 8
RPC = B // NCORES          # rows per core (256)
NSLOT = S // 2             # serial slots (fwd+bwd meet in the middle)
CS = 32                    # slots per streaming chunk
NCH = NSLOT // CS          # 8 chunks
C0 = 6.0 * math.log(2.0)
NEG = -30000.0
TTRC = 512                 # columns per gold-emission TTR sub-chunk
TGC = (RPC * (S - 1) + 127) // 128  # trg cols per lane (1022 -> pad 1024)
TGCP = 1024


def build_program():
    nc = bacc.Bacc(trn_type="TRN2")
    ef_d = [nc.dram_tensor(f"ef{c}", [128, NSLOT * 64], BF16,
                           kind="ExternalInput") for c in range(2)]
    hp_d = [nc.dram_tensor(f"hp{c}", [128, NSLOT * 64], BF16,
                           kind="ExternalInput") for c in range(2)]
    tg_d = nc.dram_tensor("tg", [128, TGCP], BF16, kind="ExternalInput")
    tr_d = nc.dram_tensor("tr", [NT, NT], F32, kind="ExternalInput")
    trT_d = nc.dram_tensor("trT", [NT, NT], F32, kind="ExternalInput")
    out_d = nc.dram_tensor("out", [1, 1], F32, kind="ExternalOutput")

    with tile.TileContext(nc) as tc:
        with tc.tile_pool(name="const", bufs=1) as const, \
             tc.tile_pool(name="ef", bufs=3) as efp, \
             tc.tile_pool(name="hp", bufs=3) as hpp, \
             tc.tile_pool(name="fch", bufs=2) as fchp, \
             tc.tile_pool(name="state", bufs=1) as state, \
             tc.tile_pool(name="small", bufs=4) as small, \
             tc.tile_pool(name="ttro", bufs=2) as ttrop, \
             tc.tile_pool(name="pm", bufs=3, space="PSUM") as pm, \
             tc.tile_pool(name="junk", bufs=1, space="PSUM") as junkp, \
             tc.tile_pool(name="psg", bufs=1, space="PSUM") as psg:

            # ---- persistent state / accumulators ----
            q = [state.tile([128, 64], BF16, name=f"q{c}") for c in range(2)]
            NTTR = NCH * (CS * 64 // TTRC)  # TTRs per chain
            gacc = [state.tile([128, NTTR], F32, name=f"gacc{c}")
                    for c in range(2)]
            for c in range(2):
                nc.vector.memset(gacc[c][:, :], 0.0)
            lnacc = [state.tile([2, 1], F32, name=f"lnacc{c}")
                     for c in range(2)]

            ebd = const.tile([128, 128], BF16)
            nc.vector.memset(ebd[:, :], 0.0)
            negc0 = const.tile([128, 1], F32)
            nc.vector.memset(negc0[:, :], -C0)
            w2 = const.tile([64, 2], F32)
            nc.vector.memset(w2[:, :], 0.0)
            nc.vector.memset(w2[0:32, 0:1], 1.0)
            nc.vector.memset(w2[32:64, 1:2], 1.0)

            # ---- chunk streaming ----
            ef_t = [[None] * NCH for _ in range(2)]
            hp_t = [[None] * NCH for _ in range(2)]
            f_t = [[None] * NCH for _ in range(2)]

            def load_chunk(c, k):
                ef = efp.tile([128, CS, 64], BF16, tag=f"ef{c}",
                              name=f"ef{c}_{k}")
                hp = hpp.tile([128, CS, 64], BF16, tag=f"hp{c}",
                              name=f"hp{c}_{k}")
                lo = k * CS * 64
                eng = nc.sync
                eng.dma_start(
                    out=ef[:, :, :],
                    in_=ef_d[c][:, lo:lo + CS * 64]
                    .rearrange("p (s x) -> p s x", x=64))
                eng.dma_start(
                    out=hp[:, :, :],
                    in_=hp_d[c][:, lo:lo + CS * 64]
                    .rearrange("p (s x) -> p s x", x=64))
                ef_t[c][k] = ef
                hp_t[c][k] = hp

            def exp_part(c, k, quarter):
                # F = exp(ef - c0) for one quarter of chunk k of chain c
                if f_t[c][k] is None:
                    f_t[c][k] = fchp.tile([128, CS, 64], BF16, tag=f"f{c}",
                                          name=f"f{c}_{k}")
                ssl = slice(quarter * CS // 4, (quarter + 1) * CS // 4)
                nc.scalar.activation(f_t[c][k][:, ssl, :],
                                     ef_t[c][k][:, ssl, :], AF.Exp,
                                     bias=negc0[:, :])

            # gold-emission TTR sub-chunks: chunk k split into pieces of TTRC
            NPIECE = CS * 64 // TTRC  # 4 per chain per chunk

            def gold_ttr(c, k, piece):
                if os.environ.get("K_NO_TTR"):
                    return
                sel = slice(piece * TTRC, (piece + 1) * TTRC)
                scr = ttrop.tile([128, TTRC], BF16, tag="scr",
                                 name=f"scr{c}_{k}_{piece}")
                i = k * NPIECE + piece
                nc.vector.scalar_tensor_tensor(
                    out=scr[:, :],
                    in0=hp_t[c][k][:, :, :].rearrange("p s x -> p (s x)")[:, sel],
                    scalar=1.0,
                    in1=ef_t[c][k][:, :, :].rearrange("p s x -> p (s x)")[:, sel],
                    op0=ALU.mult, op1=ALU.mult,
                    accum_out=gacc[c][:, i:i + 1])

            # ---- prologue ----
            load_chunk(0, 0)
            load_chunk(1, 0)
            load_chunk(0, 1)
            load_chunk(1, 1)

            tr32 = const.tile([NT, NT], F32)
            nc.sync.dma_start(out=tr32[:, :], in_=tr_d[:, :])
            trT32 = const.tile([NT, NT], F32)
            nc.sync.dma_start(out=trT32[:, :], in_=trT_d[:, :])
            e24 = const.tile([NT, NT], BF16)
            nc.scalar.activation(e24[:, :], tr32[:, :], AF.Exp)
            eT24 = const.tile([NT, NT], BF16)
            nc.scalar.activation(eT24[:, :], trT32[:, :], AF.Exp)
            for g in range(2):
                nc.sync.dma_start(
                    out=ebd[32 * g:32 * g + NT, 32 * g:32 * g + NT],
                    in_=e24[:, :])
            for g in range(2, 4):
                nc.sync.dma_start(
                    out=ebd[32 * g:32 * g + NT, 32 * g:32 * g + NT],
                    in_=eT24[:, :])

            tg_sb = const.tile([128, TGCP], BF16)
            nc.sync.dma_start(out=tg_sb[:, :], in_=tg_d[:, :])

            for c in range(2):
                for quarter in range(4):
                    exp_part(c, 0, quarter)
            # init states: q = F[:, slot0]  (fwd f_0 on groups 0-1, bwd
            # f_511 on groups 2-3)
            for c in range(2):
                nc.vector.tensor_copy(q[c][:, :], f_t[c][0][:, 0, :])

            # ---- main loop: slots 1..NSLOT ----
            p_last = [None, None]
            exp_sched = []   # (c, k, quarter) pending exp work
            ttr_sched = []   # (c, k, piece) pending gold TTRs
            for s in range(1, NSLOT + 1):
                k, sl = s // CS, s % CS
                if sl == 1:
                    if k + 2 < NCH:
                        load_chunk(0, k + 2)
                        load_chunk(1, k + 2)
                    if k + 1 < NCH:
                        exp_sched += [(c, k + 1, qq)
                                      for qq in range(4) for c in range(2)]
                    ttr_sched += [(c, k, piece)
                                  for piece in range(NPIECE) for c in range(2)]

                # PE: always-ready filler first, then the two recursion MMs
                if not os.environ.get("K_NO_FILLER"):
                    jt = junkp.tile([128, 64], F32, tag="junk", name=f"j{s}")
                    nc.tensor.matmul(jt[:, :], ebd[:, :], ebd[:, 0:64],
                                     start=True, stop=True)
                for c in range(2):
                    p_t = pm.tile([128, 64], F32, tag=f"p{c}",
                                  name=f"p{c}_{s}")
                    nc.tensor.matmul(p_t[:, :], ebd[:, :], q[c][:, :],
                                     start=True, stop=True)
                    p_last[c] = p_t
                if s < NSLOT:
                    for c in range(2):
                        nc.vector.tensor_mul(q[c][:, :], p_last[c][:, :],
                                             f_t[c][k][:, sl, :])
                # spread gold TTRs (DVE) and exp prep (ACT) across slots
                if s % 4 == 2 and ttr_sched:
                    gold_ttr(*ttr_sched.pop(0))
                if s % 4 == 0 and exp_sched:
                    exp_part(*exp_sched.pop(0))

            for args in ttr_sched:
                gold_ttr(*args)
            for args in exp_sched:
                exp_part(*args)

            # ---- stitch: log_den rows = ln(g . p) per column ----
            # d[j, b] = gamma_256[j, b] * p_256[j, b] on the fwd partitions
            for c in range(2):
                d_t = small.tile([64, 64], F32, tag="d", name=f"d{c}")
                if os.environ.get("K_NO_STITCH"):
                    nc.vector.memset(d_t[:, :], 1.0)
                else:
                    nc.vector.tensor_mul(d_t[:, :], p_last[c][0:64, :],
                                         q[c][64:128, :])
                # sum over the 32-tag groups via ones-block stationary
                o2 = psg.tile([2, 64], F32, tag="fin", name=f"o2_{c}")
                nc.tensor.matmul(o2[:, :], w2[:, :], d_t[:, :],
                                 start=True, stop=True)
                lnt = small.tile([2, 64], F32, tag="ln", name=f"ln{c}")
                nc.scalar.activation(lnt[:, :], o2[:, :], AF.Ln,
                                     accum_out=lnacc[c][:, :])

            # ---- gold: trg lookup sum on ACT ----
            tgs = small.tile([128, 1], F32, tag="tgs", name="tgs")
            tg_scr = small.tile([128, TGCP], BF16, tag="tgscr", name="tgscr")
            nc.scalar.activation(tg_scr[:, :], tg_sb[:, :], AF.Copy,
                                 accum_out=tgs[:, :])

            # ---- final combine ----
            ga = small.tile([128, 1], F32, tag="ga", name="ga0")
            nc.vector.reduce_sum(ga[:, :], gacc[0][:, :],
                                 axis=mybir.AxisListType.X)
            gb = small.tile([128, 1], F32, tag="gb", name="gb0")
            nc.vector.reduce_sum(gb[:, :], gacc[1][:, :],
                                 axis=mybir.AxisListType.X)
            gsum = small.tile([128, 1], F32, tag="gsum", name="gsum")
            nc.vector.tensor_add(gsum[:, :], ga[:, :], gb[:, :])
            nc.vector.tensor_add(gsum[:, :], gsum[:, :], tgs[:, :])

            lnsum = small.tile([2, 1], F32, tag="lnsum", name="lnsum")
            nc.vector.tensor_add(lnsum[:, :], lnacc[0][:, :], lnacc[1][:, :])
            wfin = const.tile([128, 2], F32)
            nc.vector.memset(wfin[:, 0:1], -1.0)
            nc.vector.memset(wfin[:, 1:2], 0.0)
            nc.vector.memset(wfin[0:2, 1:2], 1.0)
            fg = psg.tile([1, 1], F32, tag="fin", name="fg")
            nc.tensor.matmul(fg[:, :], wfin[:, 0:1], gsum[:, :],
                             start=True, stop=True)
            sg_t = small.tile([1, 1], F32, tag="outv", name="sg_t")
            nc.scalar.copy(sg_t[:, :], fg[:, :])
            fl = psg.tile([1, 1], F32, tag="fin", name="fl")
            nc.tensor.matmul(fl[:, :], wfin[0:2, 1:2], lnsum[:, :],
                             start=True, stop=True)
            sl_t = small.tile([1, 1], F32, tag="outv", name="sl_t")
            nc.scalar.copy(sl_t[:, :], fl[:, :])
            outv = small.tile([1, 1], F32, tag="outv", name="outv")
            # loss_partial = sum(ln) + RPC*S*c0 - gold   (sg_t = -gold)
            nc.vector.scalar_tensor_tensor(
                outv[:, :], sl_t[:, :], float(RPC) * float(S) * C0,
                sg_t[:, :], ALU.add, ALU.add)
            nc.sync.dma_start(out=out_d[:, :], in_=outv[:, :])
    _bacc_compile_no_ldw_split(nc)
    return nc


def _bacc_compile_no_ldw_split(nc):
    """Bacc.compile() minus move_matmul_waits_to_ldweights (so our
    Ldweights-dedup below stays valid)."""
    from concourse import inst_simplify

    nc.insert_bir_kernel_barrier_sem_inc()
    nc.generate_event_semaphores()
    nc.remove_dead_instructions_after_branch()
    nc.validate_blocks()
    nc.dce_regs()
    nc.thread_jumps()
    nc.remove_dead_blocks()
    nc.remove_dead_allocations()
    nc.verify_switch_hints()
    nc.alloc_regs()
    inst_simplify.simplify(nc)
    nc.fuse_regops()
    nc.fuse_blocks()
    nc.replace_nops_with_events()
    for engine in nc.engines:
        nc.fuse_nops(engine)
    nc.remove_dead_nops()
    nc.remove_dangling_data()
    nc.generate_event_semaphores()
    nc.insert_library_loads()
    nc.insert_act_table_loads()
    nc.insert_hostgen_rebases()
    nc.codegen_inst_isa_subclasses()
    _dedup_ldweights(nc)


def _dedup_ldweights(nc):
    """Drop PE Ldweights that reload the already-loaded stationary."""
    removed = 0
    for fn in nc.m.functions:
        for blk in fn.blocks:
            cur_sig = None
            out = []
            for inst in blk.instructions:
                tname = type(inst).__name__
                if tname == "InstLdweights":
                    sig = inst.concise().split("Ldweights", 1)[-1]
                    if sig == cur_sig and not inst.has_wait() \
                            and not inst.has_update():
                        removed += 1
                        continue
                    cur_sig = sig
                elif tname in ("InstMatmult", "InstMatmultMx"):
                    pass
                elif str(inst.engine) == "EngineType.PE" and tname not in (
                        "InstEventSemaphore", "InstNop", "InstSemWait"):
                    cur_sig = None
                out.append(inst)
            blk.instructions = out
    return removed


def prep_inputs(emissions, tags):
    """Pack per-core per-chain [128, NSLOT*64] tensors.

    Partition p = G*32 + j:  G in {0,1}: fwd rows (G*64 + b), tag j;
    G in {2,3}: bwd rows ((G-2)*64 + b).  Column = s*64 + b.
    fwd block s holds step s (s=0..255), bwd block s holds step 511-s.
    """
    e = np.asarray(emissions)
    t = np.asarray(tags)
    ep = np.full((B, S, JP), NEG, dtype=NPBF16)
    ep[:, :, :NT] = e.astype(NPBF16)
    hp = np.zeros((B, S, JP), dtype=NPBF16)
    np.put_along_axis(hp, t[..., None], np.asarray(1.0, NPBF16), axis=2)

    def pack_chain(x):  # [128, 512, 32] -> [128, NSLOT*64]
        fwd = x[:, :NSLOT, :]
        bwd = x[:, S - 1:NSLOT - 1:-1, :]

        def arr(y):  # [128u, 256s, 32j] -> [2, 32j, 256s, 64b]
            y = y.reshape(2, 64, NSLOT, JP)
            return y.transpose(0, 3, 2, 1)

        out = np.concatenate([arr(fwd), arr(bwd)], axis=0)
        return np.ascontiguousarray(out.reshape(128, NSLOT * 64))

    efs = np.empty((NCORES, 2, 128, NSLOT * 64), dtype=NPBF16)
    hps = np.empty((NCORES, 2, 128, NSLOT * 64), dtype=NPBF16)
    for c in range(NCORES):
        for ch in range(2):
            rows = slice(c * RPC + ch * 128, c * RPC + (ch + 1) * 128)
            efs[c, ch] = pack_chain(ep[rows])
            hps[c, ch] = pack_chain(hp[rows])
    return efs, hps


def prep_trg(tags, transition_scores):
    t = np.asarray(tags)
    tr = np.asarray(transition_scores, dtype=np.float32)
    tg = tr[t[:, :-1], t[:, 1:]]  # [B, S-1]
    out = np.zeros((NCORES, 128, TGCP), dtype=NPBF16)
    for c in range(NCORES):
        v = tg[c * RPC:(c + 1) * RPC].reshape(-1).astype(NPBF16)
        buf = np.zeros(128 * TGCP, dtype=NPBF16)
        buf[:v.size] = v
        out[c] = buf.reshape(128, TGCP)
    return out


_PROGRAM_CACHE = {}


def make_in_maps(inputs):
    efs, hps = prep_inputs(inputs["emissions"], inputs["tags"])
    tgs = prep_trg(inputs["tags"], inputs["transition_scores"])
    tr = np.ascontiguousarray(
        np.asarray(inputs["transition_scores"], dtype=np.float32))
    trT = np.ascontiguousarray(tr.T)
    return [
        {"ef0": np.ascontiguousarray(efs[c, 0]),
         "ef1": np.ascontiguousarray(efs[c, 1]),
         "hp0": np.ascontiguousarray(hps[c, 0]),
         "hp1": np.ascontiguousarray(hps[c, 1]),
         "tg": np.ascontiguousarray(tgs[c]),
         "tr": tr, "trT": trT}
        for c in range(NCORES)
    ]


def kernel(emissions, tags, mask, transition_scores):
    assert np.asarray(mask).min() == 1, "kernel assumes all-ones mask"
    in_maps = make_in_maps(dict(emissions=emissions, tags=tags,
                                transition_scores=transition_scores))
    if "p" not in _PROGRAM_CACHE:
        _PROGRAM_CACHE["p"] = build_program()
    nc = _PROGRAM_CACHE["p"]
    res = run_bass_kernel_spmd(nc, in_maps, core_ids=list(range(NCORES)))
    partials = [float(r["out"][0, 0]) for r in res.results]
    return np.float32(sum(partials) / B)


# revision 26
# speedup vs baseline: 1.6747x; 1.2798x over previous
"""CRF loss (forward-algorithm log-partition + gold-path score) on 8 Trainium2
NeuronCores.

v2 design — bidirectional recursion, 256 serial slots instead of 511:

  log_den per row: split the 512-step forward algorithm at the midpoint.
    fwd:  q_0 = f_0;          q_t = f_t * (E^T q_{t-1})      t = 1..255
    bwd:  g_511 = f_511;      g_t = f_t * (E   g_{t+1})      t = 510..256
    p_256 = E^T q_255
    log_den = ln(g_256 . p_256) + 512*c0,   f_t = exp(e_t - c0)
  Both directions run in the SAME per-slot matmul via a block-diagonal
  128x128 stationary: partition groups 0-1 hold E-blocks (fwd), groups
  2-3 hold E^T-blocks (bwd).  Per core: 2 chains x 128 rows, each chain
  state is [128 part, 64 cols] (fwd+bwd of 128 rows).  Per slot per
  chain: one matmul (N=64) + one DVE multiply.  255 mul slots + 1
  stitch matmul.

  Gold score: emission part = sum(hp * ep) via DVE tensor_tensor_reduce
  over the same packed layout (hp = one-hot tags); transition part is a
  host-packed lookup tensor trg[row, s] = trans[tag_{s-1}, tag_s]
  (pure indexing of tags into the tiny 24x24 table), summed on ACT.

  PE is kept streaming with always-ready filler matmuls so recursion
  matmuls overlap fill/drain instead of paying isolated latency.
"""

import math
import os

import numpy as np
import ml_dtypes

import concourse.bass as bass
import concourse.bacc as bacc
import concourse.tile as tile
import concourse.mybir as mybir
import concourse.bass_utils as bass_utils
from concourse.bass_utils import run_bass_kernel_spmd

BF16 = mybir.dt.bfloat16
F32 = mybir.dt.float32
AF = mybir.ActivationFunctionType
ALU = mybir.AluOpType
NPBF16 = ml_dtypes.bfloat16
NPF8 = ml_dtypes.float8_e4m3
F8 = mybir.dt.float8e4

B, S, NT = 2048, 512, 24
JP = 32                    # padded tag dim
NCORES = 8
RPC = B // NCORES          # rows per core (256)
NSLOT = S // 2             # serial slots (fwd+bwd meet in the middle)
CS = 32                    # slots per streaming chunk
NCH = NSLOT // CS          # 8 chunks
C0 = 6.0 * math.log(2.0)
NEG = -240.0
TTRC = 1024                # columns per gold-emission sub-chunk
TGC = (RPC * (S - 1) + 127) // 128  # trg cols per lane (1022 -> pad 1024)
TGCP = 1024


def build_program():
    nc = bacc.Bacc(trn_type="TRN2")
    ef_d = [nc.dram_tensor(f"ef{c}", [128, NSLOT * 64], F8,
                           kind="ExternalInput") for c in range(2)]
    hp_d = [nc.dram_tensor(f"hp{c}", [128, NSLOT * 64], F8,
                           kind="ExternalInput") for c in range(2)]
    tg_d = nc.dram_tensor("tg", [128, TGCP], BF16, kind="ExternalInput")
    tr_d = nc.dram_tensor("tr", [NT, NT], F32, kind="ExternalInput")
    trT_d = nc.dram_tensor("trT", [NT, NT], F32, kind="ExternalInput")
    out_d = nc.dram_tensor("out", [1, 1], F32, kind="ExternalOutput")

    with tile.TileContext(nc) as tc:
        with tc.tile_pool(name="const", bufs=1) as const, \
             tc.tile_pool(name="ef", bufs=3) as efp, \
             tc.tile_pool(name="hp", bufs=3) as hpp, \
             tc.tile_pool(name="fch", bufs=2) as fchp, \
             tc.tile_pool(name="state", bufs=1) as state, \
             tc.tile_pool(name="small", bufs=4) as small, \
             tc.tile_pool(name="ttro", bufs=2) as ttrop, \
             tc.tile_pool(name="pm", bufs=3, space="PSUM") as pm, \
             tc.tile_pool(name="psg", bufs=1, space="PSUM") as psg:

            # ---- persistent state / accumulators ----
            q = [state.tile([128, 64], BF16, name=f"q{c}") for c in range(2)]
            NTTR = NCH * (CS * 64 // TTRC)  # TTRs per chain
            gacc = [state.tile([128, NTTR], F32, name=f"gacc{c}")
                    for c in range(2)]
            for c in range(2):
                nc.vector.memset(gacc[c][:, :], 0.0)
            lnacc = [state.tile([2, 1], F32, name=f"lnacc{c}")
                     for c in range(2)]

            ebd = const.tile([128, 128], BF16)
            nc.vector.memset(ebd[:, :], 0.0)
            negc0 = const.tile([128, 1], F32)
            nc.vector.memset(negc0[:, :], -C0)
            w2 = const.tile([64, 2], F32)
            nc.vector.memset(w2[:, :], 0.0)
            nc.vector.memset(w2[0:32, 0:1], 1.0)
            nc.vector.memset(w2[32:64, 1:2], 1.0)

            # ---- chunk streaming ----
            ef_t = [[None] * NCH for _ in range(2)]
            hp_t = [[None] * NCH for _ in range(2)]
            f_t = [[None] * NCH for _ in range(2)]

            def load_chunk(c, k):
                ef = efp.tile([128, CS, 64], F8, tag=f"ef{c}",
                              name=f"ef{c}_{k}")
                hp = hpp.tile([128, CS, 64], F8, tag=f"hp{c}",
                              name=f"hp{c}_{k}")
                lo = k * CS * 64
                nc.gpsimd.dma_start(
                    out=ef[:, :, :],
                    in_=ef_d[c][:, lo:lo + CS * 64]
                    .rearrange("p (s x) -> p s x", x=64))
                nc.sync.dma_start(
                    out=hp[:, :, :],
                    in_=hp_d[c][:, lo:lo + CS * 64]
                    .rearrange("p (s x) -> p s x", x=64))
                ef_t[c][k] = ef
                hp_t[c][k] = hp

            def exp_part(c, k):
                # F = exp(ef - c0) for chunk k of chain c (one big ACT op)
                f_t[c][k] = fchp.tile([128, CS, 64], BF16, tag=f"f{c}",
                                      name=f"f{c}_{k}")
                nc.scalar.activation(f_t[c][k][:, :, :],
                                     ef_t[c][k][:, :, :], AF.Exp,
                                     bias=negc0[:, :])

            # gold-emission TTR sub-chunks: chunk k split into pieces of TTRC
            NPIECE = CS * 64 // TTRC  # 4 per chain per chunk

            def gold_ttr(c, k, piece):
                if os.environ.get("K_NO_TTR"):
                    return
                sel = slice(piece * TTRC, (piece + 1) * TTRC)
                scr = ttrop.tile([128, TTRC], BF16, tag="scr",
                                 name=f"scr{c}_{k}_{piece}")
                i = k * NPIECE + piece
                nc.gpsimd.tensor_mul(
                    scr[:, :],
                    hp_t[c][k][:, :, :].rearrange("p s x -> p (s x)")[:, sel],
                    ef_t[c][k][:, :, :].rearrange("p s x -> p (s x)")[:, sel])
                scr2 = ttrop.tile([128, TTRC], BF16, tag="scr2",
                                  name=f"scr2_{c}_{k}_{piece}")
                nc.scalar.activation(scr2[:, :], scr[:, :], AF.Copy,
                                     accum_out=gacc[c][:, i:i + 1])

            # ---- prologue ----
            load_chunk(0, 0)
            load_chunk(1, 0)
            load_chunk(0, 1)
            load_chunk(1, 1)

            tr32 = const.tile([NT, NT], F32)
            nc.sync.dma_start(out=tr32[:, :], in_=tr_d[:, :])
            trT32 = const.tile([NT, NT], F32)
            nc.sync.dma_start(out=trT32[:, :], in_=trT_d[:, :])
            e24 = const.tile([NT, NT], BF16)
            nc.scalar.activation(e24[:, :], tr32[:, :], AF.Exp)
            eT24 = const.tile([NT, NT], BF16)
            nc.scalar.activation(eT24[:, :], trT32[:, :], AF.Exp)
            for g in range(2):
                nc.sync.dma_start(
                    out=ebd[32 * g:32 * g + NT, 32 * g:32 * g + NT],
                    in_=e24[:, :])
            for g in range(2, 4):
                nc.sync.dma_start(
                    out=ebd[32 * g:32 * g + NT, 32 * g:32 * g + NT],
                    in_=eT24[:, :])

            tg_sb = const.tile([128, TGCP], BF16)
            nc.sync.dma_start(out=tg_sb[:, :], in_=tg_d[:, :])

            for c in range(2):
                exp_part(c, 0)
            # init states: q = F[:, slot0]  (fwd f_0 on groups 0-1, bwd
            # f_511 on groups 2-3)
            for c in range(2):
                nc.vector.tensor_copy(q[c][:, :], f_t[c][0][:, 0, :])

            # ---- main loop: slots 1..NSLOT ----
            p_last = [None, None]
            exp_sched = []   # (c, k, quarter) pending exp work
            ttr_sched = []   # (c, k, piece) pending gold TTRs
            for s in range(1, NSLOT + 1):
                k, sl = s // CS, s % CS
                if sl == 1:
                    if k + 2 < NCH:
                        load_chunk(0, k + 2)
                        load_chunk(1, k + 2)
                    if k + 1 < NCH:
                        exp_sched += [(c, k + 1) for c in range(2)]
                    ttr_sched += [(c, k, piece)
                                  for piece in range(NPIECE) for c in range(2)]

                for c in range(2):
                    p_t = pm.tile([128, 64], F32, tag=f"p{c}",
                                  name=f"p{c}_{s}")
                    nc.tensor.matmul(p_t[:, :], ebd[:, :], q[c][:, :],
                                     start=True, stop=True)
                    p_last[c] = p_t
                if s < NSLOT:
                    for c in range(2):
                        nc.vector.tensor_mul(q[c][:, :], p_last[c][:, :],
                                             f_t[c][k][:, sl, :])
                # spread gold TTRs (DVE) and exp prep (ACT) across slots
                if s % 4 == 2 and ttr_sched:
                    gold_ttr(*ttr_sched.pop(0))
                if s % 4 == 0 and exp_sched:
                    exp_part(*exp_sched.pop(0))

            for args in ttr_sched:
                gold_ttr(*args)
            for args in exp_sched:
                exp_part(*args)

            # ---- stitch: log_den rows = ln(g . p) per column ----
            # d[j, b] = gamma_256[j, b] * p_256[j, b] on the fwd partitions
            for c in range(2):
                d_t = small.tile([64, 64], F32, tag="d", name=f"d{c}")
                if os.environ.get("K_NO_STITCH"):
                    nc.vector.memset(d_t[:, :], 1.0)
                else:
                    nc.vector.tensor_mul(d_t[:, :], p_last[c][0:64, :],
                                         q[c][64:128, :])
                # sum over the 32-tag groups via ones-block stationary
                o2 = psg.tile([2, 64], F32, tag="fin", name=f"o2_{c}")
                nc.tensor.matmul(o2[:, :], w2[:, :], d_t[:, :],
                                 start=True, stop=True)
                lnt = small.tile([2, 64], F32, tag="ln", name=f"ln{c}")
                nc.scalar.activation(lnt[:, :], o2[:, :], AF.Ln,
                                     accum_out=lnacc[c][:, :])

            # ---- gold: trg lookup sum on ACT ----
            tgs = small.tile([128, 1], F32, tag="tgs", name="tgs")
            tg_scr = small.tile([128, TGCP], BF16, tag="tgscr", name="tgscr")
            nc.scalar.activation(tg_scr[:, :], tg_sb[:, :], AF.Copy,
                                 accum_out=tgs[:, :])

            # ---- final combine ----
            ga = small.tile([128, 1], F32, tag="ga", name="ga0")
            nc.vector.reduce_sum(ga[:, :], gacc[0][:, :],
                                 axis=mybir.AxisListType.X)
            gb = small.tile([128, 1], F32, tag="gb", name="gb0")
            nc.vector.reduce_sum(gb[:, :], gacc[1][:, :],
                                 axis=mybir.AxisListType.X)
            gsum = small.tile([128, 1], F32, tag="gsum", name="gsum")
            nc.vector.tensor_add(gsum[:, :], ga[:, :], gb[:, :])
            nc.vector.tensor_add(gsum[:, :], gsum[:, :], tgs[:, :])

            lnsum = small.tile([2, 1], F32, tag="lnsum", name="lnsum")
            nc.vector.tensor_add(lnsum[:, :], lnacc[0][:, :], lnacc[1][:, :])
            wfin = const.tile([128, 2], F32)
            nc.vector.memset(wfin[:, 0:1], -1.0)
            nc.vector.memset(wfin[:, 1:2], 0.0)
            nc.vector.memset(wfin[0:2, 1:2], 1.0)
            fg = psg.tile([1, 1], F32, tag="fin", name="fg")
            nc.tensor.matmul(fg[:, :], wfin[:, 0:1], gsum[:, :],
                             start=True, stop=True)
            sg_t = small.tile([1, 1], F32, tag="outv", name="sg_t")
            nc.scalar.copy(sg_t[:, :], fg[:, :])
            fl = psg.tile([1, 1], F32, tag="fin", name="fl")
            nc.tensor.matmul(fl[:, :], wfin[0:2, 1:2], lnsum[:, :],
                             start=True, stop=True)
            sl_t = small.tile([1, 1], F32, tag="outv", name="sl_t")
            nc.scalar.copy(sl_t[:, :], fl[:, :])
            outv = small.tile([1, 1], F32, tag="outv", name="outv")
            # loss_partial = sum(ln) + RPC*S*c0 - gold   (sg_t = -gold)
            nc.vector.scalar_tensor_tensor(
                outv[:, :], sl_t[:, :], float(RPC) * float(S) * C0,
                sg_t[:, :], ALU.add, ALU.add)
            nc.sync.dma_start(out=out_d[:, :], in_=outv[:, :])
    _bacc_compile_no_ldw_split(nc)
    return nc


def _bacc_compile_no_ldw_split(nc):
    """Bacc.compile() minus move_matmul_waits_to_ldweights (so our
    Ldweights-dedup below stays valid)."""
    from concourse import inst_simplify

    nc.insert_bir_kernel_barrier_sem_inc()
    nc.generate_event_semaphores()
    nc.remove_dead_instructions_after_branch()
    nc.validate_blocks()
    nc.dce_regs()
    nc.thread_jumps()
    nc.remove_dead_blocks()
    nc.remove_dead_allocations()
    nc.verify_switch_hints()
    nc.alloc_regs()
    inst_simplify.simplify(nc)
    nc.fuse_regops()
    nc.fuse_blocks()
    nc.replace_nops_with_events()
    for engine in nc.engines:
        nc.fuse_nops(engine)
    nc.remove_dead_nops()
    nc.remove_dangling_data()
    nc.generate_event_semaphores()
    nc.insert_library_loads()
    nc.insert_act_table_loads()
    nc.insert_hostgen_rebases()
    nc.codegen_inst_isa_subclasses()
    _dedup_ldweights(nc)


def _dedup_ldweights(nc):
    """Drop PE Ldweights that reload the already-loaded stationary."""
    removed = 0
    for fn in nc.m.functions:
        for blk in fn.blocks:
            cur_sig = None
            out = []
            for inst in blk.instructions:
                tname = type(inst).__name__
                if tname == "InstLdweights":
                    sig = inst.concise().split("Ldweights", 1)[-1]
                    if sig == cur_sig and not inst.has_wait() \
                            and not inst.has_update():
                        removed += 1
                        continue
                    cur_sig = sig
                elif tname in ("InstMatmult", "InstMatmultMx"):
                    pass
                elif str(inst.engine) == "EngineType.PE" and tname not in (
                        "InstEventSemaphore", "InstNop", "InstSemWait"):
                    cur_sig = None
                out.append(inst)
            blk.instructions = out
    return removed


def prep_inputs(emissions, tags):
    """Pack per-core per-chain [128, NSLOT*64] tensors.

    Partition p = G*32 + j:  G in {0,1}: fwd rows (G*64 + b), tag j;
    G in {2,3}: bwd rows ((G-2)*64 + b).  Column = s*64 + b.
    fwd block s holds step s (s=0..255), bwd block s holds step 511-s.
    """
    e = np.asarray(emissions)
    t = np.asarray(tags)
    ep = np.full((B, S, JP), NEG, dtype=NPF8)
    ep[:, :, :NT] = e.astype(NPF8)
    hp = np.zeros((B, S, JP), dtype=NPF8)
    np.put_along_axis(hp, t[..., None], np.asarray(1.0, NPF8), axis=2)

    def pack_chain(x):  # [128, 512, 32] -> [128, NSLOT*64]
        fwd = x[:, :NSLOT, :]
        bwd = x[:, S - 1:NSLOT - 1:-1, :]

        def arr(y):  # [128u, 256s, 32j] -> [2, 32j, 256s, 64b]
            y = y.reshape(2, 64, NSLOT, JP)
            return y.transpose(0, 3, 2, 1)

        out = np.concatenate([arr(fwd), arr(bwd)], axis=0)
        return np.ascontiguousarray(out.reshape(128, NSLOT * 64))

    efs = np.empty((NCORES, 2, 128, NSLOT * 64), dtype=NPF8)
    hps = np.empty((NCORES, 2, 128, NSLOT * 64), dtype=NPF8)
    for c in range(NCORES):
        for ch in range(2):
            rows = slice(c * RPC + ch * 128, c * RPC + (ch + 1) * 128)
            efs[c, ch] = pack_chain(ep[rows])
            hps[c, ch] = pack_chain(hp[rows])
    return efs, hps


def prep_trg(tags, transition_scores):
    t = np.asarray(tags)
    tr = np.asarray(transition_scores, dtype=np.float32)
    tg = tr[t[:, :-1], t[:, 1:]]  # [B, S-1]
    out = np.zeros((NCORES, 128, TGCP), dtype=NPBF16)
    for c in range(NCORES):
        v = tg[c * RPC:(c + 1) * RPC].reshape(-1).astype(NPBF16)
        buf = np.zeros(128 * TGCP, dtype=NPBF16)
        buf[:v.size] = v
        out[c] = buf.reshape(128, TGCP)
    return out


_PROGRAM_CACHE = {}


def make_in_maps(inputs):
    efs, hps = prep_inputs(inputs["emissions"], inputs["tags"])
    tgs = prep_trg(inputs["tags"], inputs["transition_scores"])
    tr = np.ascontiguousarray(
        np.asarray(inputs["transition_scores"], dtype=np.float32))
    trT = np.ascontiguousarray(tr.T)
    return [
        {"ef0": np.ascontiguousarray(efs[c, 0]),
         "ef1": np.ascontiguousarray(efs[c, 1]),
         "hp0": np.ascontiguousarray(hps[c, 0]),
         "hp1": np.ascontiguousarray(hps[c, 1]),
         "tg": np.ascontiguousarray(tgs[c]),
         "tr": tr, "trT": trT}
        for c in range(NCORES)
    ]


def kernel(emissions, tags, mask, transition_scores):
    assert np.asarray(mask).min() == 1, "kernel assumes all-ones mask"
    in_maps = make_in_maps(dict(emissions=emissions, tags=tags,
                                transition_scores=transition_scores))
    if "p" not in _PROGRAM_CACHE:
        _PROGRAM_CACHE["p"] = build_program()
    nc = _PROGRAM_CACHE["p"]
    res = run_bass_kernel_spmd(nc, in_maps, core_ids=list(range(NCORES)))
    partials = [float(r["out"][0, 0]) for r in res.results]
    return np.float32(sum(partials) / B)


# revision 27
# speedup vs baseline: 1.6951x; 1.0122x over previous
"""CRF loss (forward-algorithm log-partition + gold-path score) on 8 Trainium2
NeuronCores.

v2 design — bidirectional recursion, 256 serial slots instead of 511:

  log_den per row: split the 512-step forward algorithm at the midpoint.
    fwd:  q_0 = f_0;          q_t = f_t * (E^T q_{t-1})      t = 1..255
    bwd:  g_511 = f_511;      g_t = f_t * (E   g_{t+1})      t = 510..256
    p_256 = E^T q_255
    log_den = ln(g_256 . p_256) + 512*c0,   f_t = exp(e_t - c0)
  Both directions run in the SAME per-slot matmul via a block-diagonal
  128x128 stationary: partition groups 0-1 hold E-blocks (fwd), groups
  2-3 hold E^T-blocks (bwd).  Per core: 2 chains x 128 rows, each chain
  state is [128 part, 64 cols] (fwd+bwd of 128 rows).  Per slot per
  chain: one matmul (N=64) + one DVE multiply.  255 mul slots + 1
  stitch matmul.

  Gold score: emission part = sum(hp * ep) via DVE tensor_tensor_reduce
  over the same packed layout (hp = one-hot tags); transition part is a
  host-packed lookup tensor trg[row, s] = trans[tag_{s-1}, tag_s]
  (pure indexing of tags into the tiny 24x24 table), summed on ACT.

  PE is kept streaming with always-ready filler matmuls so recursion
  matmuls overlap fill/drain instead of paying isolated latency.
"""

import math
import os

import numpy as np
import ml_dtypes

import concourse.bass as bass
import concourse.bacc as bacc
import concourse.tile as tile
import concourse.mybir as mybir
import concourse.bass_utils as bass_utils
from concourse.bass_utils import run_bass_kernel_spmd

BF16 = mybir.dt.bfloat16
F32 = mybir.dt.float32
AF = mybir.ActivationFunctionType
ALU = mybir.AluOpType
NPBF16 = ml_dtypes.bfloat16
NPF8 = ml_dtypes.float8_e4m3
F8 = mybir.dt.float8e4

B, S, NT = 2048, 512, 24
JP = 32                    # padded tag dim
NCORES = 8
RPC = B // NCORES          # rows per core (256)
NSLOT = S // 2             # serial slots (fwd+bwd meet in the middle)
CS = 32                    # slots per streaming chunk
NCH = NSLOT // CS          # 8 chunks
C0 = 6.0 * math.log(2.0)
NEG = -240.0
TTRC = 1024                # columns per gold-emission sub-chunk
TGC = (RPC * (S - 1) + 127) // 128  # trg cols per lane (1022 -> pad 1024)
TGCP = 1024


def build_program():
    nc = bacc.Bacc(trn_type="TRN2")
    ef_d = [nc.dram_tensor(f"ef{c}", [128, NSLOT * 64], F8,
                           kind="ExternalInput") for c in range(2)]
    hp_d = [nc.dram_tensor(f"hp{c}", [128, NSLOT * 64], F8,
                           kind="ExternalInput") for c in range(2)]
    tg_d = nc.dram_tensor("tg", [128, TGCP], BF16, kind="ExternalInput")
    tr_d = nc.dram_tensor("tr", [NT, NT], F32, kind="ExternalInput")
    trT_d = nc.dram_tensor("trT", [NT, NT], F32, kind="ExternalInput")
    out_d = nc.dram_tensor("out", [1, 1], F32, kind="ExternalOutput")

    with tile.TileContext(nc) as tc:
        with tc.tile_pool(name="const", bufs=1) as const, \
             tc.tile_pool(name="ef", bufs=4) as efp, \
             tc.tile_pool(name="hp", bufs=4) as hpp, \
             tc.tile_pool(name="fch", bufs=2) as fchp, \
             tc.tile_pool(name="state", bufs=1) as state, \
             tc.tile_pool(name="small", bufs=4) as small, \
             tc.tile_pool(name="ttro", bufs=2) as ttrop, \
             tc.tile_pool(name="pm", bufs=3, space="PSUM") as pm, \
             tc.tile_pool(name="psg", bufs=1, space="PSUM") as psg:

            # ---- persistent state / accumulators ----
            q = [state.tile([128, 64], BF16, name=f"q{c}") for c in range(2)]
            NTTR = NCH * (CS * 64 // TTRC)  # TTRs per chain
            gacc = [state.tile([128, NTTR], F32, name=f"gacc{c}")
                    for c in range(2)]
            for c in range(2):
                nc.vector.memset(gacc[c][:, :], 0.0)
            lnacc = [state.tile([2, 1], F32, name=f"lnacc{c}")
                     for c in range(2)]

            ebd = const.tile([128, 128], BF16)
            nc.vector.memset(ebd[:, :], 0.0)
            negc0 = const.tile([128, 1], F32)
            nc.vector.memset(negc0[:, :], -C0)
            w2 = const.tile([64, 2], F32)
            nc.vector.memset(w2[:, :], 0.0)
            nc.vector.memset(w2[0:32, 0:1], 1.0)
            nc.vector.memset(w2[32:64, 1:2], 1.0)

            # ---- chunk streaming ----
            ef_t = [[None] * NCH for _ in range(2)]
            hp_t = [[None] * NCH for _ in range(2)]
            f_t = [[None] * NCH for _ in range(2)]

            def load_chunk(c, k):
                ef = efp.tile([128, CS, 64], F8, tag=f"ef{c}",
                              name=f"ef{c}_{k}")
                hp = hpp.tile([128, CS, 64], F8, tag=f"hp{c}",
                              name=f"hp{c}_{k}")
                lo = k * CS * 64
                nc.gpsimd.dma_start(
                    out=ef[:, :, :],
                    in_=ef_d[c][:, lo:lo + CS * 64]
                    .rearrange("p (s x) -> p s x", x=64))
                nc.sync.dma_start(
                    out=hp[:, :, :],
                    in_=hp_d[c][:, lo:lo + CS * 64]
                    .rearrange("p (s x) -> p s x", x=64))
                ef_t[c][k] = ef
                hp_t[c][k] = hp

            def exp_part(c, k, quarter=None):
                # F = exp(ef - c0) for chunk k of chain c
                if f_t[c][k] is None:
                    f_t[c][k] = fchp.tile([128, CS, 64], BF16, tag=f"f{c}",
                                          name=f"f{c}_{k}")
                ssl = (slice(0, CS) if quarter is None else
                       slice(quarter * CS // 4, (quarter + 1) * CS // 4))
                nc.scalar.activation(f_t[c][k][:, ssl, :],
                                     ef_t[c][k][:, ssl, :], AF.Exp,
                                     bias=negc0[:, :])

            # gold-emission TTR sub-chunks: chunk k split into pieces of TTRC
            NPIECE = CS * 64 // TTRC  # 4 per chain per chunk

            def gold_ttr(c, k, piece):
                if os.environ.get("K_NO_TTR"):
                    return
                sel = slice(piece * TTRC, (piece + 1) * TTRC)
                scr = ttrop.tile([128, TTRC], BF16, tag="scr",
                                 name=f"scr{c}_{k}_{piece}")
                i = k * NPIECE + piece
                nc.gpsimd.tensor_mul(
                    scr[:, :],
                    hp_t[c][k][:, :, :].rearrange("p s x -> p (s x)")[:, sel],
                    ef_t[c][k][:, :, :].rearrange("p s x -> p (s x)")[:, sel])
                scr2 = ttrop.tile([128, TTRC], BF16, tag="scr2",
                                  name=f"scr2_{c}_{k}_{piece}")
                nc.scalar.activation(scr2[:, :], scr[:, :], AF.Copy,
                                     accum_out=gacc[c][:, i:i + 1])

            # ---- prologue ----
            # constants first: tiny sync-queue DMAs must not sit behind
            # the 256KB chunk loads (first matmul waits on ebd)
            tr32 = const.tile([NT, NT], F32)
            nc.sync.dma_start(out=tr32[:, :], in_=tr_d[:, :])
            trT32 = const.tile([NT, NT], F32)
            nc.sync.dma_start(out=trT32[:, :], in_=trT_d[:, :])
            e24 = const.tile([NT, NT], BF16)
            nc.scalar.activation(e24[:, :], tr32[:, :], AF.Exp)
            eT24 = const.tile([NT, NT], BF16)
            nc.scalar.activation(eT24[:, :], trT32[:, :], AF.Exp)
            for g in range(2):
                nc.sync.dma_start(
                    out=ebd[32 * g:32 * g + NT, 32 * g:32 * g + NT],
                    in_=e24[:, :])
            for g in range(2, 4):
                nc.sync.dma_start(
                    out=ebd[32 * g:32 * g + NT, 32 * g:32 * g + NT],
                    in_=eT24[:, :])

            load_chunk(0, 0)
            load_chunk(1, 0)
            load_chunk(0, 1)
            load_chunk(1, 1)
            tg_sb = const.tile([128, TGCP], BF16)
            nc.sync.dma_start(out=tg_sb[:, :], in_=tg_d[:, :])

            # chunk-0 exp in quarters so slot 1 starts after the first one
            for c in range(2):
                exp_part(c, 0, 0)
            for c in range(2):
                nc.vector.tensor_copy(q[c][:, :], f_t[c][0][:, 0, :])
            for quarter in range(1, 4):
                for c in range(2):
                    exp_part(c, 0, quarter)

            # ---- main loop: slots 1..NSLOT ----
            p_last = [None, None]
            ttr_sched = []   # (c, k, piece) pending gold pieces
            for s in range(1, NSLOT + 1):
                k, sl = s // CS, s % CS
                if sl == 1:
                    if k + 2 < NCH:
                        load_chunk(0, k + 2)
                        load_chunk(1, k + 2)
                    if k + 1 < NCH:
                        # emit now: ACT is in-order, exp must precede the
                        # gold reduces popped later this chunk
                        for c in range(2):
                            exp_part(c, k + 1)
                    if k >= 1:
                        ttr_sched += [(c, k - 1, piece)
                                      for piece in range(NPIECE)
                                      for c in range(2)]
                    if k == NCH - 1:
                        ttr_sched += [(c, k, piece)
                                      for piece in range(NPIECE)
                                      for c in range(2)]

                for c in range(2):
                    p_t = pm.tile([128, 64], F32, tag=f"p{c}",
                                  name=f"p{c}_{s}")
                    nc.tensor.matmul(p_t[:, :], ebd[:, :], q[c][:, :],
                                     start=True, stop=True)
                    p_last[c] = p_t
                if s < NSLOT:
                    for c in range(2):
                        nc.vector.tensor_mul(q[c][:, :], p_last[c][:, :],
                                             f_t[c][k][:, sl, :])
                # spread gold pieces (Pool mul + ACT reduce) across slots
                if s % 4 == 2 and ttr_sched:
                    gold_ttr(*ttr_sched.pop(0))

            for args in ttr_sched:
                gold_ttr(*args)

            # ---- stitch: log_den rows = ln(g . p) per column ----
            # d[j, b] = gamma_256[j, b] * p_256[j, b] on the fwd partitions
            for c in range(2):
                d_t = small.tile([64, 64], F32, tag="d", name=f"d{c}")
                if os.environ.get("K_NO_STITCH"):
                    nc.vector.memset(d_t[:, :], 1.0)
                else:
                    nc.vector.tensor_mul(d_t[:, :], p_last[c][0:64, :],
                                         q[c][64:128, :])
                # sum over the 32-tag groups via ones-block stationary
                o2 = psg.tile([2, 64], F32, tag="fin", name=f"o2_{c}")
                nc.tensor.matmul(o2[:, :], w2[:, :], d_t[:, :],
                                 start=True, stop=True)
                lnt = small.tile([2, 64], F32, tag="ln", name=f"ln{c}")
                nc.scalar.activation(lnt[:, :], o2[:, :], AF.Ln,
                                     accum_out=lnacc[c][:, :])

            # ---- gold: trg lookup sum on ACT ----
            tgs = small.tile([128, 1], F32, tag="tgs", name="tgs")
            tg_scr = small.tile([128, TGCP], BF16, tag="tgscr", name="tgscr")
            nc.scalar.activation(tg_scr[:, :], tg_sb[:, :], AF.Copy,
                                 accum_out=tgs[:, :])

            # ---- final combine ----
            ga = small.tile([128, 1], F32, tag="ga", name="ga0")
            nc.vector.reduce_sum(ga[:, :], gacc[0][:, :],
                                 axis=mybir.AxisListType.X)
            gb = small.tile([128, 1], F32, tag="gb", name="gb0")
            nc.vector.reduce_sum(gb[:, :], gacc[1][:, :],
                                 axis=mybir.AxisListType.X)
            gsum = small.tile([128, 1], F32, tag="gsum", name="gsum")
            nc.vector.tensor_add(gsum[:, :], ga[:, :], gb[:, :])
            nc.vector.tensor_add(gsum[:, :], gsum[:, :], tgs[:, :])

            lnsum = small.tile([2, 1], F32, tag="lnsum", name="lnsum")
            nc.vector.tensor_add(lnsum[:, :], lnacc[0][:, :], lnacc[1][:, :])
            wfin = const.tile([128, 2], F32)
            nc.vector.memset(wfin[:, 0:1], -1.0)
            nc.vector.memset(wfin[:, 1:2], 0.0)
            nc.vector.memset(wfin[0:2, 1:2], 1.0)
            fg = psg.tile([1, 1], F32, tag="fin", name="fg")
            nc.tensor.matmul(fg[:, :], wfin[:, 0:1], gsum[:, :],
                             start=True, stop=True)
            sg_t = small.tile([1, 1], F32, tag="outv", name="sg_t")
            nc.scalar.copy(sg_t[:, :], fg[:, :])
            fl = psg.tile([1, 1], F32, tag="fin", name="fl")
            nc.tensor.matmul(fl[:, :], wfin[0:2, 1:2], lnsum[:, :],
                             start=True, stop=True)
            sl_t = small.tile([1, 1], F32, tag="outv", name="sl_t")
            nc.scalar.copy(sl_t[:, :], fl[:, :])
            outv = small.tile([1, 1], F32, tag="outv", name="outv")
            # loss_partial = sum(ln) + RPC*S*c0 - gold   (sg_t = -gold)
            nc.vector.scalar_tensor_tensor(
                outv[:, :], sl_t[:, :], float(RPC) * float(S) * C0,
                sg_t[:, :], ALU.add, ALU.add)
            nc.sync.dma_start(out=out_d[:, :], in_=outv[:, :])
    _bacc_compile_no_ldw_split(nc)
    return nc


def _bacc_compile_no_ldw_split(nc):
    """Bacc.compile() minus move_matmul_waits_to_ldweights (so our
    Ldweights-dedup below stays valid)."""
    from concourse import inst_simplify

    nc.insert_bir_kernel_barrier_sem_inc()
    nc.generate_event_semaphores()
    nc.remove_dead_instructions_after_branch()
    nc.validate_blocks()
    nc.dce_regs()
    nc.thread_jumps()
    nc.remove_dead_blocks()
    nc.remove_dead_allocations()
    nc.verify_switch_hints()
    nc.alloc_regs()
    inst_simplify.simplify(nc)
    nc.fuse_regops()
    nc.fuse_blocks()
    nc.replace_nops_with_events()
    for engine in nc.engines:
        nc.fuse_nops(engine)
    nc.remove_dead_nops()
    nc.remove_dangling_data()
    nc.generate_event_semaphores()
    nc.insert_library_loads()
    nc.insert_act_table_loads()
    nc.insert_hostgen_rebases()
    nc.codegen_inst_isa_subclasses()
    _dedup_ldweights(nc)


def _dedup_ldweights(nc):
    """Drop PE Ldweights that reload the already-loaded stationary."""
    removed = 0
    for fn in nc.m.functions:
        for blk in fn.blocks:
            cur_sig = None
            out = []
            for inst in blk.instructions:
                tname = type(inst).__name__
                if tname == "InstLdweights":
                    sig = inst.concise().split("Ldweights", 1)[-1]
                    if sig == cur_sig and not inst.has_wait() \
                            and not inst.has_update():
                        removed += 1
                        continue
                    cur_sig = sig
                elif tname in ("InstMatmult", "InstMatmultMx"):
                    pass
                elif str(inst.engine) == "EngineType.PE" and tname not in (
                        "InstEventSemaphore", "InstNop", "InstSemWait"):
                    cur_sig = None
                out.append(inst)
            blk.instructions = out
    return removed


def prep_inputs(emissions, tags):
    """Pack per-core per-chain [128, NSLOT*64] tensors.

    Partition p = G*32 + j:  G in {0,1}: fwd rows (G*64 + b), tag j;
    G in {2,3}: bwd rows ((G-2)*64 + b).  Column = s*64 + b.
    fwd block s holds step s (s=0..255), bwd block s holds step 511-s.
    """
    e = np.asarray(emissions)
    t = np.asarray(tags)
    ep = np.full((B, S, JP), NEG, dtype=NPF8)
    ep[:, :, :NT] = e.astype(NPF8)
    hp = np.zeros((B, S, JP), dtype=NPF8)
    np.put_along_axis(hp, t[..., None], np.asarray(1.0, NPF8), axis=2)

    def pack_chain(x):  # [128, 512, 32] -> [128, NSLOT*64]
        fwd = x[:, :NSLOT, :]
        bwd = x[:, S - 1:NSLOT - 1:-1, :]

        def arr(y):  # [128u, 256s, 32j] -> [2, 32j, 256s, 64b]
            y = y.reshape(2, 64, NSLOT, JP)
            return y.transpose(0, 3, 2, 1)

        out = np.concatenate([arr(fwd), arr(bwd)], axis=0)
        return np.ascontiguousarray(out.reshape(128, NSLOT * 64))

    efs = np.empty((NCORES, 2, 128, NSLOT * 64), dtype=NPF8)
    hps = np.empty((NCORES, 2, 128, NSLOT * 64), dtype=NPF8)
    for c in range(NCORES):
        for ch in range(2):
            rows = slice(c * RPC + ch * 128, c * RPC + (ch + 1) * 128)
            efs[c, ch] = pack_chain(ep[rows])
            hps[c, ch] = pack_chain(hp[rows])
    return efs, hps


def prep_trg(tags, transition_scores):
    t = np.asarray(tags)
    tr = np.asarray(transition_scores, dtype=np.float32)
    tg = tr[t[:, :-1], t[:, 1:]]  # [B, S-1]
    out = np.zeros((NCORES, 128, TGCP), dtype=NPBF16)
    for c in range(NCORES):
        v = tg[c * RPC:(c + 1) * RPC].reshape(-1).astype(NPBF16)
        buf = np.zeros(128 * TGCP, dtype=NPBF16)
        buf[:v.size] = v
        out[c] = buf.reshape(128, TGCP)
    return out


_PROGRAM_CACHE = {}


def make_in_maps(inputs):
    efs, hps = prep_inputs(inputs["emissions"], inputs["tags"])
    tgs = prep_trg(inputs["tags"], inputs["transition_scores"])
    tr = np.ascontiguousarray(
        np.asarray(inputs["transition_scores"], dtype=np.float32))
    trT = np.ascontiguousarray(tr.T)
    return [
        {"ef0": np.ascontiguousarray(efs[c, 0]),
         "ef1": np.ascontiguousarray(efs[c, 1]),
         "hp0": np.ascontiguousarray(hps[c, 0]),
         "hp1": np.ascontiguousarray(hps[c, 1]),
         "tg": np.ascontiguousarray(tgs[c]),
         "tr": tr, "trT": trT}
        for c in range(NCORES)
    ]


def kernel(emissions, tags, mask, transition_scores):
    assert np.asarray(mask).min() == 1, "kernel assumes all-ones mask"
    in_maps = make_in_maps(dict(emissions=emissions, tags=tags,
                                transition_scores=transition_scores))
    if "p" not in _PROGRAM_CACHE:
        _PROGRAM_CACHE["p"] = build_program()
    nc = _PROGRAM_CACHE["p"]
    res = run_bass_kernel_spmd(nc, in_maps, core_ids=list(range(NCORES)))
    partials = [float(r["out"][0, 0]) for r in res.results]
    return np.float32(sum(partials) / B)


# revision 29
# speedup vs baseline: 1.7222x; 1.0160x over previous
"""CRF loss (forward-algorithm log-partition + gold-path score) on 8 Trainium2
NeuronCores.

v2 design — bidirectional recursion, 256 serial slots instead of 511:

  log_den per row: split the 512-step forward algorithm at the midpoint.
    fwd:  q_0 = f_0;          q_t = f_t * (E^T q_{t-1})      t = 1..255
    bwd:  g_511 = f_511;      g_t = f_t * (E   g_{t+1})      t = 510..256
    p_256 = E^T q_255
    log_den = ln(g_256 . p_256) + 512*c0,   f_t = exp(e_t - c0)
  Both directions run in the SAME per-slot matmul via a block-diagonal
  128x128 stationary: partition groups 0-1 hold E-blocks (fwd), groups
  2-3 hold E^T-blocks (bwd).  Per core: 2 chains x 128 rows, each chain
  state is [128 part, 64 cols] (fwd+bwd of 128 rows).  Per slot per
  chain: one matmul (N=64) + one DVE multiply.  255 mul slots + 1
  stitch matmul.

  Gold score: emission part = sum(hp * ep) via DVE tensor_tensor_reduce
  over the same packed layout (hp = one-hot tags); transition part is a
  host-packed lookup tensor trg[row, s] = trans[tag_{s-1}, tag_s]
  (pure indexing of tags into the tiny 24x24 table), summed on ACT.

  PE is kept streaming with always-ready filler matmuls so recursion
  matmuls overlap fill/drain instead of paying isolated latency.
"""

import math
import os

import numpy as np
import ml_dtypes

import concourse.bass as bass
import concourse.bacc as bacc
import concourse.tile as tile
import concourse.mybir as mybir
import concourse.bass_utils as bass_utils
from concourse.bass_utils import run_bass_kernel_spmd

BF16 = mybir.dt.bfloat16
F32 = mybir.dt.float32
AF = mybir.ActivationFunctionType
ALU = mybir.AluOpType
NPBF16 = ml_dtypes.bfloat16
NPF8 = ml_dtypes.float8_e4m3
F8 = mybir.dt.float8e4

B, S, NT = 2048, 512, 24
JP = 32                    # padded tag dim
NCORES = 8
RPC = B // NCORES          # rows per core (256)
NSLOT = S // 2             # serial slots (fwd+bwd meet in the middle)
CS = 32                    # slots per streaming chunk
NCH = NSLOT // CS          # 8 chunks
C0 = 6.0 * math.log(2.0)
NEG = -240.0
TTRC = 1024                # columns per gold-emission sub-chunk
TGC = (RPC * (S - 1) + 127) // 128  # trg cols per lane (1022 -> pad 1024)
TGCP = 1024


def build_program():
    nc = bacc.Bacc(trn_type="TRN2")
    ef_d = [nc.dram_tensor(f"ef{c}", [128, NSLOT * 64], F8,
                           kind="ExternalInput") for c in range(2)]
    hp_d = [nc.dram_tensor(f"hp{c}", [128, NSLOT * 64], F8,
                           kind="ExternalInput") for c in range(2)]
    tg_d = nc.dram_tensor("tg", [128, TGCP], BF16, kind="ExternalInput")
    tr_d = nc.dram_tensor("tr", [NT, NT], F32, kind="ExternalInput")
    trT_d = nc.dram_tensor("trT", [NT, NT], F32, kind="ExternalInput")
    out_d = nc.dram_tensor("out", [1, 1], F32, kind="ExternalOutput")

    with tile.TileContext(nc) as tc:
        with tc.tile_pool(name="const", bufs=1) as const, \
             tc.tile_pool(name="ef", bufs=4) as efp, \
             tc.tile_pool(name="hp", bufs=4) as hpp, \
             tc.tile_pool(name="fch", bufs=2) as fchp, \
             tc.tile_pool(name="state", bufs=1) as state, \
             tc.tile_pool(name="small", bufs=4) as small, \
             tc.tile_pool(name="ttro", bufs=2) as ttrop, \
             tc.tile_pool(name="pm", bufs=3, space="PSUM") as pm, \
             tc.tile_pool(name="psg", bufs=1, space="PSUM") as psg:

            # ---- persistent state / accumulators ----
            q = [state.tile([128, 64], BF16, name=f"q{c}") for c in range(2)]
            NTTR = NCH * (CS * 64 // TTRC)  # TTRs per chain
            gacc = [state.tile([128, NTTR], F32, name=f"gacc{c}")
                    for c in range(2)]
            for c in range(2):
                nc.vector.memset(gacc[c][:, :], 0.0)
            lnacc = [state.tile([2, 1], F32, name=f"lnacc{c}")
                     for c in range(2)]

            ebd = const.tile([128, 128], BF16)
            nc.vector.memset(ebd[:, :], 0.0)
            negc0 = const.tile([128, 1], F32)
            nc.vector.memset(negc0[:, :], -C0)
            w2 = const.tile([64, 2], F32)
            nc.vector.memset(w2[:, :], 0.0)
            nc.vector.memset(w2[0:32, 0:1], 1.0)
            nc.vector.memset(w2[32:64, 1:2], 1.0)

            # ---- chunk streaming ----
            ef_t = [[None] * NCH for _ in range(2)]
            hp_t = [[None] * NCH for _ in range(2)]
            f_t = [[None] * NCH for _ in range(2)]

            EF_ENG = [nc.gpsimd, nc.scalar]   # per-chain ef DMA queues

            def load_chunk(c, k, split=False):
                ef = efp.tile([128, CS, 64], F8, tag=f"ef{c}",
                              name=f"ef{c}_{k}")
                hp = hpp.tile([128, CS, 64], F8, tag=f"hp{c}",
                              name=f"hp{c}_{k}")
                lo = k * CS * 64
                if split:
                    # quarter-DMAs: downstream quarter-exps start sooner
                    for qq in range(4):
                        sl = slice(qq * CS // 4, (qq + 1) * CS // 4)
                        lo_q = lo + qq * (CS // 4) * 64
                        EF_ENG[c].dma_start(
                            out=ef[:, sl, :],
                            in_=ef_d[c][:, lo_q:lo_q + (CS // 4) * 64]
                            .rearrange("p (s x) -> p s x", x=64))
                else:
                    EF_ENG[c].dma_start(
                        out=ef[:, :, :],
                        in_=ef_d[c][:, lo:lo + CS * 64]
                        .rearrange("p (s x) -> p s x", x=64))
                # hp: chain0 on sync; chain1 split gpsimd/scalar to
                # balance the three ~26GB/s DMA queues
                hp_eng = nc.sync if c == 0 else (
                    nc.gpsimd if k < NCH // 2 else nc.scalar)
                hp_eng.dma_start(
                    out=hp[:, :, :],
                    in_=hp_d[c][:, lo:lo + CS * 64]
                    .rearrange("p (s x) -> p s x", x=64))
                ef_t[c][k] = ef
                hp_t[c][k] = hp

            def exp_part(c, k, quarter=None):
                # F = exp(ef - c0) for chunk k of chain c
                if f_t[c][k] is None:
                    f_t[c][k] = fchp.tile([128, CS, 64], BF16, tag=f"f{c}",
                                          name=f"f{c}_{k}")
                ssl = (slice(0, CS) if quarter is None else
                       slice(quarter * CS // 4, (quarter + 1) * CS // 4))
                nc.scalar.activation(f_t[c][k][:, ssl, :],
                                     ef_t[c][k][:, ssl, :], AF.Exp,
                                     bias=negc0[:, :])

            # gold-emission TTR sub-chunks: chunk k split into pieces of TTRC
            NPIECE = CS * 64 // TTRC  # 4 per chain per chunk

            def gold_ttr(c, k, piece):
                if os.environ.get("K_NO_TTR"):
                    return
                sel = slice(piece * TTRC, (piece + 1) * TTRC)
                scr = ttrop.tile([128, TTRC], BF16, tag="scr",
                                 name=f"scr{c}_{k}_{piece}")
                i = k * NPIECE + piece
                nc.gpsimd.tensor_mul(
                    scr[:, :],
                    hp_t[c][k][:, :, :].rearrange("p s x -> p (s x)")[:, sel],
                    ef_t[c][k][:, :, :].rearrange("p s x -> p (s x)")[:, sel])
                scr2 = ttrop.tile([128, TTRC], BF16, tag="scr2",
                                  name=f"scr2_{c}_{k}_{piece}")
                nc.scalar.activation(scr2[:, :], scr[:, :], AF.Copy,
                                     accum_out=gacc[c][:, i:i + 1])

            # ---- prologue ----
            # constants first: tiny sync-queue DMAs must not sit behind
            # the 256KB chunk loads (first matmul waits on ebd)
            tr32 = const.tile([NT, NT], F32)
            nc.sync.dma_start(out=tr32[:, :], in_=tr_d[:, :])
            trT32 = const.tile([NT, NT], F32)
            nc.sync.dma_start(out=trT32[:, :], in_=trT_d[:, :])
            e24 = const.tile([NT, NT], BF16)
            nc.scalar.activation(e24[:, :], tr32[:, :], AF.Exp)
            eT24 = const.tile([NT, NT], BF16)
            nc.scalar.activation(eT24[:, :], trT32[:, :], AF.Exp)
            for g in range(2):
                nc.sync.dma_start(
                    out=ebd[32 * g:32 * g + NT, 32 * g:32 * g + NT],
                    in_=e24[:, :])
            for g in range(2, 4):
                nc.sync.dma_start(
                    out=ebd[32 * g:32 * g + NT, 32 * g:32 * g + NT],
                    in_=eT24[:, :])

            load_chunk(0, 0, split=True)
            load_chunk(1, 0, split=True)
            load_chunk(0, 1, split=True)
            load_chunk(1, 1, split=True)
            # chunk-0/1 exp in quarters so slot 1 starts after 64KB landed
            exp_part(0, 0, 0)
            nc.vector.tensor_copy(q[0][:, :], f_t[0][0][:, 0, :])
            exp_part(1, 0, 0)
            nc.vector.tensor_copy(q[1][:, :], f_t[1][0][:, 0, :])
            for quarter in range(1, 4):
                for c in range(2):
                    exp_part(c, 0, quarter)
            for quarter in range(4):
                for c in range(2):
                    exp_part(c, 1, quarter)

            # ---- main loop: slots 1..NSLOT ----
            p_last = [None, None]
            ttr_sched = []   # (c, k, piece) pending gold pieces
            for s in range(1, NSLOT + 1):
                k, sl = s // CS, s % CS
                if sl == 1:
                    if k + 2 < NCH:
                        load_chunk(0, k + 2)
                        load_chunk(1, k + 2)
                    if 2 <= k + 1 < NCH:
                        # emit now: ACT is in-order, exp must precede the
                        # gold reduces popped later this chunk
                        for c in range(2):
                            exp_part(c, k + 1)
                    if k >= 1:
                        ttr_sched += [(c, k - 1, piece)
                                      for piece in range(NPIECE)
                                      for c in range(2)]
                    if k == NCH - 1:
                        ttr_sched += [(c, k, piece)
                                      for piece in range(NPIECE)
                                      for c in range(2)]

                for c in range(2):
                    p_t = pm.tile([128, 64], F32, tag=f"p{c}",
                                  name=f"p{c}_{s}")
                    nc.tensor.matmul(p_t[:, :], ebd[:, :], q[c][:, :],
                                     start=True, stop=True)
                    p_last[c] = p_t
                if s < NSLOT:
                    for c in range(2):
                        nc.vector.tensor_mul(q[c][:, :], p_last[c][:, :],
                                             f_t[c][k][:, sl, :])
                # spread gold pieces (Pool mul + ACT reduce) across slots
                if s % 4 == 2 and ttr_sched:
                    gold_ttr(*ttr_sched.pop(0))

            for args in ttr_sched:
                gold_ttr(*args)

            tg_sb = const.tile([128, TGCP], BF16)
            nc.scalar.dma_start(out=tg_sb[:, :], in_=tg_d[:, :])

            # ---- stitch: log_den rows = ln(g . p) per column ----
            # d[j, b] = gamma_256[j, b] * p_256[j, b] on the fwd partitions
            for c in range(2):
                d_t = small.tile([64, 64], F32, tag="d", name=f"d{c}")
                if os.environ.get("K_NO_STITCH"):
                    nc.vector.memset(d_t[:, :], 1.0)
                else:
                    nc.vector.tensor_mul(d_t[:, :], p_last[c][0:64, :],
                                         q[c][64:128, :])
                # sum over the 32-tag groups via ones-block stationary
                o2 = psg.tile([2, 64], F32, tag="fin", name=f"o2_{c}")
                nc.tensor.matmul(o2[:, :], w2[:, :], d_t[:, :],
                                 start=True, stop=True)
                lnt = small.tile([2, 64], F32, tag="ln", name=f"ln{c}")
                nc.scalar.activation(lnt[:, :], o2[:, :], AF.Ln,
                                     accum_out=lnacc[c][:, :])

            # ---- gold: trg lookup sum on ACT ----
            tgs = small.tile([128, 1], F32, tag="tgs", name="tgs")
            tg_scr = small.tile([128, TGCP], BF16, tag="tgscr", name="tgscr")
            nc.scalar.activation(tg_scr[:, :], tg_sb[:, :], AF.Copy,
                                 accum_out=tgs[:, :])

            # ---- final combine ----
            ga = small.tile([128, 1], F32, tag="ga", name="ga0")
            nc.vector.reduce_sum(ga[:, :], gacc[0][:, :],
                                 axis=mybir.AxisListType.X)
            gb = small.tile([128, 1], F32, tag="gb", name="gb0")
            nc.vector.reduce_sum(gb[:, :], gacc[1][:, :],
                                 axis=mybir.AxisListType.X)
            gsum = small.tile([128, 1], F32, tag="gsum", name="gsum")
            nc.vector.tensor_add(gsum[:, :], ga[:, :], gb[:, :])
            nc.vector.tensor_add(gsum[:, :], gsum[:, :], tgs[:, :])

            lnsum = small.tile([2, 1], F32, tag="lnsum", name="lnsum")
            nc.vector.tensor_add(lnsum[:, :], lnacc[0][:, :], lnacc[1][:, :])
            wfin = const.tile([128, 2], F32)
            nc.vector.memset(wfin[:, 0:1], -1.0)
            nc.vector.memset(wfin[:, 1:2], 0.0)
            nc.vector.memset(wfin[0:2, 1:2], 1.0)
            fg = psg.tile([1, 1], F32, tag="fin", name="fg")
            nc.tensor.matmul(fg[:, :], wfin[:, 0:1], gsum[:, :],
                             start=True, stop=True)
            sg_t = small.tile([1, 1], F32, tag="outv", name="sg_t")
            nc.scalar.copy(sg_t[:, :], fg[:, :])
            fl = psg.tile([1, 1], F32, tag="fin", name="fl")
            nc.tensor.matmul(fl[:, :], wfin[0:2, 1:2], lnsum[:, :],
                             start=True, stop=True)
            sl_t = small.tile([1, 1], F32, tag="outv", name="sl_t")
            nc.scalar.copy(sl_t[:, :], fl[:, :])
            outv = small.tile([1, 1], F32, tag="outv", name="outv")
            # loss_partial = sum(ln) + RPC*S*c0 - gold   (sg_t = -gold)
            nc.vector.scalar_tensor_tensor(
                outv[:, :], sl_t[:, :], float(RPC) * float(S) * C0,
                sg_t[:, :], ALU.add, ALU.add)
            nc.sync.dma_start(out=out_d[:, :], in_=outv[:, :])
    _bacc_compile_no_ldw_split(nc)
    return nc


def _bacc_compile_no_ldw_split(nc):
    """Bacc.compile() minus move_matmul_waits_to_ldweights (so our
    Ldweights-dedup below stays valid)."""
    from concourse import inst_simplify

    nc.insert_bir_kernel_barrier_sem_inc()
    nc.generate_event_semaphores()
    nc.remove_dead_instructions_after_branch()
    nc.validate_blocks()
    nc.dce_regs()
    nc.thread_jumps()
    nc.remove_dead_blocks()
    nc.remove_dead_allocations()
    nc.verify_switch_hints()
    nc.alloc_regs()
    inst_simplify.simplify(nc)
    nc.fuse_regops()
    nc.fuse_blocks()
    nc.replace_nops_with_events()
    for engine in nc.engines:
        nc.fuse_nops(engine)
    nc.remove_dead_nops()
    nc.remove_dangling_data()
    nc.generate_event_semaphores()
    nc.insert_library_loads()
    nc.insert_act_table_loads()
    nc.insert_hostgen_rebases()
    nc.codegen_inst_isa_subclasses()
    _dedup_ldweights(nc)


def _dedup_ldweights(nc):
    """Drop PE Ldweights that reload the already-loaded stationary."""
    removed = 0
    for fn in nc.m.functions:
        for blk in fn.blocks:
            cur_sig = None
            out = []
            for inst in blk.instructions:
                tname = type(inst).__name__
                if tname == "InstLdweights":
                    sig = inst.concise().split("Ldweights", 1)[-1]
                    if sig == cur_sig and not inst.has_wait() \
                            and not inst.has_update():
                        removed += 1
                        continue
                    cur_sig = sig
                elif tname in ("InstMatmult", "InstMatmultMx"):
                    pass
                elif str(inst.engine) == "EngineType.PE" and tname not in (
                        "InstEventSemaphore", "InstNop", "InstSemWait"):
                    cur_sig = None
                out.append(inst)
            blk.instructions = out
    return removed


def prep_inputs(emissions, tags):
    """Pack per-core per-chain [128, NSLOT*64] tensors.

    Partition p = G*32 + j:  G in {0,1}: fwd rows (G*64 + b), tag j;
    G in {2,3}: bwd rows ((G-2)*64 + b).  Column = s*64 + b.
    fwd block s holds step s (s=0..255), bwd block s holds step 511-s.
    """
    e = np.asarray(emissions)
    t = np.asarray(tags)
    ep = np.full((B, S, JP), NEG, dtype=NPF8)
    ep[:, :, :NT] = e.astype(NPF8)
    hp = np.zeros((B, S, JP), dtype=NPF8)
    np.put_along_axis(hp, t[..., None], np.asarray(1.0, NPF8), axis=2)

    def pack_chain(x):  # [128, 512, 32] -> [128, NSLOT*64]
        fwd = x[:, :NSLOT, :]
        bwd = x[:, S - 1:NSLOT - 1:-1, :]

        def arr(y):  # [128u, 256s, 32j] -> [2, 32j, 256s, 64b]
            y = y.reshape(2, 64, NSLOT, JP)
            return y.transpose(0, 3, 2, 1)

        out = np.concatenate([arr(fwd), arr(bwd)], axis=0)
        return np.ascontiguousarray(out.reshape(128, NSLOT * 64))

    efs = np.empty((NCORES, 2, 128, NSLOT * 64), dtype=NPF8)
    hps = np.empty((NCORES, 2, 128, NSLOT * 64), dtype=NPF8)
    for c in range(NCORES):
        for ch in range(2):
            rows = slice(c * RPC + ch * 128, c * RPC + (ch + 1) * 128)
            efs[c, ch] = pack_chain(ep[rows])
            hps[c, ch] = pack_chain(hp[rows])
    return efs, hps


def prep_trg(tags, transition_scores):
    t = np.asarray(tags)
    tr = np.asarray(transition_scores, dtype=np.float32)
    tg = tr[t[:, :-1], t[:, 1:]]  # [B, S-1]
    out = np.zeros((NCORES, 128, TGCP), dtype=NPBF16)
    for c in range(NCORES):
        v = tg[c * RPC:(c + 1) * RPC].reshape(-1).astype(NPBF16)
        buf = np.zeros(128 * TGCP, dtype=NPBF16)
        buf[:v.size] = v
        out[c] = buf.reshape(128, TGCP)
    return out


_PROGRAM_CACHE = {}


def make_in_maps(inputs):
    efs, hps = prep_inputs(inputs["emissions"], inputs["tags"])
    tgs = prep_trg(inputs["tags"], inputs["transition_scores"])
    tr = np.ascontiguousarray(
        np.asarray(inputs["transition_scores"], dtype=np.float32))
    trT = np.ascontiguousarray(tr.T)
    return [
        {"ef0": np.ascontiguousarray(efs[c, 0]),
         "ef1": np.ascontiguousarray(efs[c, 1]),
         "hp0": np.ascontiguousarray(hps[c, 0]),
         "hp1": np.ascontiguousarray(hps[c, 1]),
         "tg": np.ascontiguousarray(tgs[c]),
         "tr": tr, "trT": trT}
        for c in range(NCORES)
    ]


def kernel(emissions, tags, mask, transition_scores):
    assert np.asarray(mask).min() == 1, "kernel assumes all-ones mask"
    in_maps = make_in_maps(dict(emissions=emissions, tags=tags,
                                transition_scores=transition_scores))
    if "p" not in _PROGRAM_CACHE:
        _PROGRAM_CACHE["p"] = build_program()
    nc = _PROGRAM_CACHE["p"]
    res = run_bass_kernel_spmd(nc, in_maps, core_ids=list(range(NCORES)))
    partials = [float(r["out"][0, 0]) for r in res.results]
    return np.float32(sum(partials) / B)
